# revision 1
# baseline (speedup 1.0000x reference)
"""Trainium2 Bass kernel for additive (tanh) attention with mask.

Computation (per batch b):
    wah    = h @ W_ah.T                             [B, H]
    e      = tanh(wah[:, None, :] + p_att_feats)    [B, M, H]
    logits = e @ w_alpha                            [B, M]
    logits = where(mask == 0, -1e9, logits)
    alpha  = softmax(logits, -1)
    att    = alpha @ att_feats                      [B, D]

Strategy: pure data-parallel over batch (8 batches / core on 8 cores).
Masked rows contribute exactly 0 to the softmax-weighted sum, so the
kernel only streams the ~50% of att_feats / p_att_feats rows with
mask==1, gathered by row index with SWDGE dma_gather, and ships both
bulk streams as bf16 (~21 MB/core vs 80 MB dense fp32).  exp() is
applied without max-subtraction (logits are bounded:
|logits| <= ||w_alpha||_1 with e in [-1,1]), masked/pad rows get an
additive -1e9 bias so their exp underflows to exactly 0, and the
normalization by 1/sum is applied once at PSUM drain time.  The
weighted-sum matmuls run bf16 x bf16 -> fp32 PSUM; exp() writes its
bf16 PE-weight tile directly (no cast pass).

Host-side work is limited to marshalling: batch->core assignment
(balanced by mask count so the SPMD gather sizes match across cores),
mask->row-index/bias tables, dtype/layout permutations of the inputs
(the wah matmul itself runs on device).

Measured on 8xNC-v3 (axon): ~82-88 us/core per full pass (paired
interleaved For_i-slope method).  Progression: f32r split streams
135 us -> bf16 split streams 94 us -> bf16 merged rows 85 us ->
+32KB SWDGE descriptor ring -> multi-packet gather descriptors
(single_packet=False): absolute per-pass ~80-88 us across slope runs
(between-run spread of the absolute estimator is ~+-5 us; within-run
paired A/B shows multipacket is ~3-8 us better than single-packet).  Real SDMA gather throughput is
descriptor-overhead bound, which the cost model underprices: merging
p+feats into one 5KB-row tensor halved the descriptor count and beat
the model-preferred schedules; a 2x descriptor-ring carveout
(dynamic_dma_scratch_size=32768) bought a further ~4 us by letting Q7
descriptor generation run a full gather ahead of the SDMA engines.
Numerics vs fp32 reference: rel-err ~2.5e-3, absmax-relative ~2.3e-3
(bf16 input quantization; f32r variants give 2.4e-4 at ~1.6x the time
via KERNEL_ATT_DTYPE/KERNEL_P_DTYPE).

Implementation notes (hard-won):
  - InstTensorTensorReduce crashes the NRT exec on this runtime; the
    logits dot-product uses the fused scalar_tensor_tensor (+accum
    row-sum), which is fine on HW.
  - float32r matmul operands must be *produced* as float32r (BIR
    verifier); DRAM tensors are declared f32r/bf16 and exp() writes the
    PE-weight tile in that dtype directly.
  - Cross-partition reduction for the softmax denominator is a DVE-only
    copy/add log-tree + 32x32 stream transpose; gpsimd
    partition_all_reduce would contend with gather descriptor
    generation on the Pool engine.
  - Phase-1 SBUF pools are opened before the phase-0 scratch pool so
    the stack allocator gives them non-overlapping addresses (otherwise
    a false overlap-dependency stalls the first gathers ~17 us).
  - wah row broadcast to 128 partitions uses a one-hot lhsT matmul
    (oh_j.T @ wah) -- no SBUF->SBUF DMA on the critical path.

Self-contained: hardcodes B=64, M=1024, RNN=1024, H=512, D=2048, 8 cores.
"""

import os

import numpy as np

import concourse.bacc as bacc
import concourse.bass as bass
import concourse.mybir as mybir
from concourse import bass_isa, library_config
from concourse.bass_utils import run_bass_kernel_spmd
from concourse.tile import TileContext

B, M, RNN, H, D = 64, 1024, 1024, 512, 2048
NCORES = 8
BL = B // NCORES  # batches per core
NEG = -1e9
F32 = mybir.dt.float32
F32R = mybir.dt.float32r
I16 = mybir.dt.int16

# Dtype of the gathered att_feats stream + PE weighted-sum matmul:
#   bf16 (default): halves the dominant DMA stream; output err ~1e-3
#   f32r: full 4-byte stream, tf32-like matmul; output err ~2e-4
#   f32:  full precision, but the PE runs at 1/4 rate
ATT_DT = os.environ.get("KERNEL_ATT_DTYPE", "bf16")
ATT_FP32 = ATT_DT == "f32"
# Dtype of the gathered p_att_feats stream (tanh input)
P_DT = os.environ.get("KERNEL_P_DTYPE", "bf16")


def _plan(mask: np.ndarray):
    """Assign batches to (core, slot) balanced by unmasked count; compute
    per-slot padded gather sizes (identical across cores - SPMD)."""
    n = mask.sum(axis=1).astype(np.int64)  # [B]
    order = np.argsort(-n, kind="stable")
    batch_of = np.empty((NCORES, BL), dtype=np.int64)
    for j in range(BL):
        for c in range(NCORES):
            batch_of[c, j] = order[j * NCORES + c]
    nbar = np.empty(BL, dtype=np.int64)
    for j in range(BL):
        mx = max(int(n[batch_of[c, j]]) for c in range(NCORES))
        nbar[j] = ((mx + 15) // 16) * 16  # multiple of 16 for idx wrap
    nch = [(int(v) + 127) // 128 for v in nbar]
    return batch_of, n, nbar, nch


def _build(nbar, nch, reps=1, bench_mode=False, loop_n=0, fsplit=2,
           ring=32768, fbufs=3, spkt=False):
    """Build the SPMD bass program (same for all cores).  reps>1 repeats
    phase 1 (benchmark amplification only; outputs are overwritten).
    bench_mode replaces the two bulk inputs (feats/p) with device-side
    zero-filled internal DRAM so per-call host transfer is tiny."""
    stot = int(sum(v // 16 for v in nbar))  # idx columns (int16)
    tch = int(sum(nch))  # total chunks (bias columns)
    soff = np.cumsum([0] + [int(v) // 16 for v in nbar])
    boff = np.cumsum([0] + list(nch))
    max_nch = max(nch)

    FATT = {"bf16": mybir.dt.bfloat16, "f32r": F32R, "f32": F32}[ATT_DT]
    nc = bacc.Bacc(
        "TRN2", target_bir_lowering=False, dynamic_dma_scratch_size=ring
    )
    # p and feats are host-concatenated row-wise into one tensor so each
    # unmasked row is ONE large gather descriptor (5KB) instead of a 1KB +
    # a 4KB one -- real SDMA throughput is descriptor-overhead sensitive.
    assert ATT_DT == P_DT or ATT_FP32 == (P_DT != "bf16")
    CW = H + D  # combined row width (elements)
    if bench_mode:
        comb_d = nc.dram_tensor("comb_i", [BL * M, CW], FATT)
    else:
        comb_d = nc.dram_tensor("comb", [BL * M, CW], FATT, kind="ExternalInput")
    # W^T and h^T arrive pre-permuted from the host (layout marshalling):
    # wt[p, rc, hh] = W[hh, rc*128+p], ht[p, rc, b] = h[b, rc*128+p].
    # f32r dram views let the PE consume them at 1 cycle/row.
    wt_d = nc.dram_tensor("wt", [128, RNN // 128, H], F32R, kind="ExternalInput")
    ht_d = nc.dram_tensor("ht", [128, RNN // 128, BL], F32R, kind="ExternalInput")
    wa_d = nc.dram_tensor("walpha", [1, H], F32R, kind="ExternalInput")
    # oh[b, j*128+p] = (b == j): one-hot lhsT used to broadcast row j of the
    # [BL, H] wah tile to all 128 partitions without any SBUF->SBUF move
    oh_d = nc.dram_tensor("oh", [BL, BL * 128], F32R, kind="ExternalInput")
    idx_d = nc.dram_tensor("idx", [128, stot], I16, kind="ExternalInput")
    bias_d = nc.dram_tensor("bias", [128, tch], F32, kind="ExternalInput")
    ones_d = nc.dram_tensor("ones", [1, 128], F32R, kind="ExternalInput")
    out_d = nc.dram_tensor("out", [BL, D], F32, kind="ExternalOutput")

    RC = RNN // 128  # 8

    with TileContext(nc) as tc:
        nc.gpsimd.load_library(library_config.mlp)
        # Pool order matters: phase-1 pools (fp/pp/wk/sm) are allocated
        # BEFORE the phase-0 scratch pool so their SBUF addresses do not
        # overlap it -- otherwise the stack allocator's overlap-dep would
        # stall the first gathers until all of phase 0 has drained.
        with (
            tc.tile_pool(name="const", bufs=1) as cp,
            tc.tile_pool(name="fp", bufs=fbufs) as fp,
            tc.tile_pool(name="lp", bufs=4) as lp,
            tc.tile_pool(name="wk", bufs=4) as wk,
            tc.tile_pool(name="sm", bufs=3) as sm,
            tc.tile_pool(name="op", bufs=2) as op,
        ):
            idx_t = cp.tile([128, stot], I16)
            nc.sync.dma_start(idx_t[:, :], idx_d[:, :])
            if bench_mode:
                # zero-fill the internal bulk tensor once (phase -1)
                with tc.tile_pool(name="fill", bufs=1) as fillp:
                    ztf = fillp.tile([128, CW], FATT)
                    nc.vector.memset(ztf[:, :], 0.0)
                    for blk in range(BL * M // 128):
                        nc.sync.dma_start(
                            comb_d[blk * 128 : (blk + 1) * 128, :], ztf[:, :]
                        )
            bias_t = cp.tile([128, tch], F32)
            nc.sync.dma_start(bias_t[:, :], bias_d[:, :])
            wahb = cp.tile([128, BL, H], F32)  # per-slot wah broadcast
            walphab = cp.tile([128, H], F32)  # w_alpha broadcast

            # ---------------- phase 0: wah = h @ W.T, broadcasts ----------
            with (
                tc.tile_pool(name="ph0", bufs=1) as p0,
                tc.tile_pool(name="ph0w", bufs=2) as p0w,
                tc.tile_pool(name="ph0ps", bufs=2, space="PSUM") as p0ps,
            ):
                ones_sb = p0.tile([1, 128], F32R)
                nc.sync.dma_start(ones_sb[:, :], ones_d[:, :])
                oh_sb = p0.tile([BL, BL * 128], F32R)
                nc.sync.dma_start(oh_sb[:, :], oh_d[:, :])
                wa_sb = p0.tile([1, H], F32R)
                nc.sync.dma_start(wa_sb[:, :], wa_d[:, :])
                wt_sb = p0.tile([128, RC, H], F32R)
                nc.sync.dma_start(wt_sb[:, :, :], wt_d[:, :, :])
                ht_sb = p0.tile([128, RC, BL], F32R)
                nc.sync.dma_start(ht_sb[:, :, :], ht_d[:, :, :])

                # wah [b, h] = sum_r h^T.T @ W^T
                ps_wah = p0ps.tile([BL, H], F32, tag="wah")
                for rc in range(RC):
                    nc.tensor.matmul(
                        ps_wah[:, :],
                        ht_sb[:, rc, :],
                        wt_sb[:, rc, :],
                        start=(rc == 0),
                        stop=(rc == RC - 1),
                    )
                wah_sb = p0.tile([BL, H], F32R)
                nc.vector.tensor_copy(wah_sb[:, :], ps_wah[:, :])
                # broadcast row j to 128 partitions: onehot_j.T @ wah_sb
                for j in range(BL):
                    pb = p0ps.tile([128, H], F32, tag="bc")
                    nc.tensor.matmul(
                        pb[:, :],
                        oh_sb[:, j * 128 : (j + 1) * 128],
                        wah_sb[:, :],
                        start=True, stop=True,
                    )
                    nc.scalar.copy(wahb[:, j, :], pb[:, :])
                pb = p0ps.tile([128, H], F32, tag="bc")
                nc.tensor.matmul(
                    pb[:, :], ones_sb[:, :], wa_sb[:, :], start=True, stop=True
                )
                nc.scalar.copy(walphab[:, :], pb[:, :])

            # ---------------- phase 1: per-slot sparse attention ----------
            def issue_f_gather(j):
                nj, cj = int(nbar[j]), nch[j]
                f_t = fp.tile([128, max_nch, CW], FATT, tag="f")
                # split the gather so the pipeline starts on the first
                # piece while the rest streams
                s0 = int(soff[j])
                per = max(1, (cj + fsplit - 1) // fsplit)
                c0 = 0
                while c0 < cj:
                    c1 = min(cj, c0 + per)
                    r0, r1 = c0 * 128, min(nj, c1 * 128)
                    nc.gpsimd.dma_gather(
                        f_t[:, c0:c1, :], comb_d[:, :],
                        idx_t[:, s0 + r0 // 16 : s0 + r1 // 16],
                        r1 - r0, r1 - r0, CW, single_packet=spkt
                    )
                    c0 = c1
                return f_t

            import contextlib

            with tc.tile_pool(name="aps", bufs=2, space="PSUM") as aps:
                loop_cm = (
                    tc.For_i(0, loop_n, 1,
                             hint_engines=tuple(mybir.ALL_ENGINES))
                    if loop_n else contextlib.nullcontext()
                )
                with loop_cm:
                  for rep in range(reps):
                    pending_f = issue_f_gather(0)
                    for j in range(BL):
                        nj, cj = int(nbar[j]), nch[j]
                        f_t = pending_f
                        if j + 1 < BL:
                            pending_f = issue_f_gather(j + 1)

                        logits = lp.tile([128, max_nch], F32, tag="lg")
                        nc.vector.memset(logits[:, :], 0.0)
                        exr = lp.tile([128, max_nch], FATT, tag="exr")
                        ps = aps.tile([1, D], F32, tag="att")
                        for c in range(cj):
                            kc = min(128, nj - c * 128)
                            e = wk.tile([128, H], F32, tag="e")
                            nc.vector.tensor_add(
                                e[:kc, :], f_t[:kc, c, 0:H], wahb[:kc, j, :]
                            )
                            nc.scalar.activation(
                                e[:kc, :], e[:kc, :], mybir.ActivationFunctionType.Tanh
                            )
                            # NOTE: InstTensorTensorReduce crashes the device
                            # (NRT exec error) on this runtime; the fused
                            # scalar_tensor_tensor (+accum row-sum) is fine.
                            tt = wk.tile([128, H], F32, tag="tt")
                            nc.vector.scalar_tensor_tensor(
                                out=tt[:kc, :],
                                in0=e[:kc, :],
                                scalar=1.0,
                                in1=walphab[:kc, :],
                                op0=mybir.AluOpType.mult,
                                op1=mybir.AluOpType.mult,
                                accum_out=logits[:kc, c : c + 1],
                            )
                            # exp(logits + bias); bias = -1e9 on masked/pad
                            # rows.  Output dtype doubles as the PE weight
                            # dtype (bf16/f32r) -- no separate cast pass.
                            nc.scalar.activation(
                                exr[:, c : c + 1],
                                logits[:, c : c + 1],
                                mybir.ActivationFunctionType.Exp,
                                bias=bias_t[:, int(boff[j]) + c : int(boff[j]) + c + 1],
                            )
                            lhsT = exr[:kc, c : c + 1]
                            for d in range(D // 512):
                                nc.tensor.matmul(
                                    ps[0:1, d * 512 : (d + 1) * 512],
                                    lhsT,
                                    f_t[:kc, c, H + d * 512 : H + (d + 1) * 512],
                                    start=(c == 0),
                                    stop=(c == cj - 1),
                                )
                        # s = sum over all rows of exm.  Partition reduction is
                        # done as a DVE-only log-tree (copy to rebase partitions
                        # + add, then a 32x32 transpose) so the Pool engine stays
                        # dedicated to gather descriptor generation.
                        rowsum = sm.tile([128, 1], F32, tag="rs")
                        nc.vector.tensor_reduce(
                            rowsum[:, :],
                            exr[:, :cj],
                            axis=mybir.AxisListType.X,
                            op=mybir.AluOpType.add,
                        )
                        c1 = sm.tile([64, 1], F32, tag="c1")
                        nc.vector.tensor_copy(c1[:, :], rowsum[64:128, :])
                        a1 = sm.tile([64, 1], F32, tag="a1")
                        nc.vector.tensor_add(a1[:, :], rowsum[0:64, :], c1[:, :])
                        c2 = sm.tile([32, 1], F32, tag="c2")
                        nc.vector.tensor_copy(c2[:, :], a1[32:64, :])
                        stg = sm.tile([32, 32], F32, tag="stg")
                        nc.vector.memset(stg[:, :], 0.0)
                        nc.vector.tensor_add(stg[:, 0:1], a1[0:32, :], c2[:, :])
                        trp = sm.tile([32, 32], F32, tag="trp")
                        nc.vector.transpose(trp[:, :], stg[:, :])
                        sv = sm.tile([1, 1], F32, tag="sv")
                        nc.vector.tensor_reduce(
                            sv[0:1, :],
                            trp[0:1, :],
                            axis=mybir.AxisListType.X,
                            op=mybir.AluOpType.add,
                        )
                        rinv = sm.tile([1, 1], F32, tag="ri")
                        nc.vector.reciprocal(rinv[:, :], sv[:, :])
                        att = op.tile([1, D], F32, tag="at")
                        nc.scalar.activation(
                            att[:, :],
                            ps[0:1, :],
                            mybir.ActivationFunctionType.Copy,
                            scale=rinv[0:1, :],
                        )
                        nc.sync.dma_start(out_d[j : j + 1, :], att[:, :])
    nc.compile()
    return nc


_CACHE: dict = {}


def _get_compiled(mask: np.ndarray):
    key = mask.tobytes()
    hit = _CACHE.get("key") == key
    if not hit:
        batch_of, n, nbar, nch = _plan(mask)
        nc = _build(nbar, nch)
        _CACHE.update(
            key=key, nc=nc, batch_of=batch_of, n=n, nbar=nbar, nch=nch
        )
    return _CACHE


def kernel(h, att_feats, att_mask, p_att_feats, W_ah, w_alpha):
    h = np.ascontiguousarray(np.asarray(h, dtype=np.float32))
    att_feats = np.ascontiguousarray(np.asarray(att_feats, dtype=np.float32))
    mask = np.asarray(att_mask).astype(np.int32)
    p_att_feats = np.ascontiguousarray(np.asarray(p_att_feats, dtype=np.float32))
    W_ah = np.ascontiguousarray(np.asarray(W_ah, dtype=np.float32))
    w_alpha = np.ascontiguousarray(np.asarray(w_alpha, dtype=np.float32))

    st = _get_compiled(mask)
    nc, batch_of, n, nbar, nch = st["nc"], st["batch_of"], st["n"], st["nbar"], st["nch"]
    stot = int(sum(int(v) // 16 for v in nbar))
    tch = int(sum(nch))
    soff = np.cumsum([0] + [int(v) // 16 for v in nbar])
    boff = np.cumsum([0] + list(nch))

    import ml_dtypes

    feats_np = {
        "bf16": ml_dtypes.bfloat16, "f32r": np.float32, "f32": np.float32
    }[ATT_DT]
    p_np = ml_dtypes.bfloat16 if P_DT == "bf16" else np.float32
    ones = np.ones((1, 128), dtype=np.float32)
    oh = np.zeros((BL, BL * 128), dtype=np.float32)
    for j in range(BL):
        oh[j, j * 128 : (j + 1) * 128] = 1.0
    wa_row = np.ascontiguousarray(w_alpha.reshape(1, H))
    # wt[p, rc, hh] = W_ah[hh, rc*128+p]
    wt_arr = np.ascontiguousarray(
        W_ah.T.reshape(RNN // 128, 128, H).transpose(1, 0, 2)
    )

    in_maps = []
    for c in range(NCORES):
        bids = batch_of[c]
        idx_arr = np.zeros((128, stot), dtype=np.int16)
        bias_arr = np.full((128, tch), NEG, dtype=np.float32)
        for j in range(BL):
            b = int(bids[j])
            nb = int(n[b])
            nj = int(nbar[j])
            rows = np.nonzero(mask[b])[0].astype(np.int64)
            assert rows.size == nb
            pad = np.zeros(nj, dtype=np.int64)
            pad[:nb] = rows + j * M
            blk = pad.reshape(nj // 16, 16).T.astype(np.int16)  # [16, nj/16]
            idx_arr[:, int(soff[j]) : int(soff[j + 1])] = np.tile(blk, (8, 1))
            # bias: 0 for valid rows (i < nb), -1e9 otherwise
            for ci in range(nch[j]):
                i0 = ci * 128
                nvalid = min(128, max(0, nb - i0))
                bias_arr[:nvalid, int(boff[j]) + ci] = 0.0
        h_l = h[bids]  # [BL, RNN]
        ht_arr = np.ascontiguousarray(
            h_l.T.reshape(RNN // 128, 128, BL).transpose(1, 0, 2)
        )
        in_maps.append(
            {
                "comb": np.concatenate(
                    [
                        p_att_feats[bids].reshape(BL * M, H).astype(p_np),
                        att_feats[bids].reshape(BL * M, D).astype(feats_np),
                    ],
                    axis=1,
                ),
                "wt": wt_arr,
                "ht": ht_arr,
                "walpha": wa_row,
                "idx": idx_arr,
                "bias": bias_arr,
                "ones": ones,
                "oh": oh,
            }
        )

    res = run_bass_kernel_spmd(nc, in_maps, core_ids=list(range(NCORES)))
    kernel._last_results = res  # for test harness introspection

    out = np.empty((B, D), dtype=np.float32)
    for c in range(NCORES):
        o = res.results[c]["out"]
        for j in range(BL):
            out[int(batch_of[c, j])] = o[j]
    return out

